# revision 1
# baseline (speedup 1.0000x reference)
"""Trainium2 Bass kernel for nn_MultiHeadAttentionBlock (B=2, L=2048, D=1024, H=16).

Sharding: 8 cores = 2 batches x 4 head-groups (4 heads each), Megatron-style.
Each core computes q/k/v projections for its 4 heads (column-sharded weights),
RoPE, attention, and a partial output projection (row-sharded w_o). The host
sums the 4 partial outputs per batch (the "all-reduce").

Layout choices (host-side prep, all free):
  - activations are pre-transposed to feature-major qT/kT/vT [1024, 2048]
    so every matmul contracts over the partition dim with contiguous DMA.
  - w_q/w_k rows are permuted per head into [even dims | odd dims] halves so
    RoPE becomes a half-rotation handled by whole-tile ops; 1/sqrt(dk) is
    folded into w_q.
  - mask is pre-transposed and sent as bf16 {0,1}; softmax is computed as
    exp(s) * m / sum(exp(s) * m)  (no max subtraction needed: |s| < ~8, so
    exp never overflows, and masked entries are exactly zeroed).
  - the softmax denominator comes from an extra all-ones column appended to V
    (attn @ [V | 1] yields both the numerator and the row sums).
"""

import contextlib
import sys

import numpy as np

sys.path.insert(0, "/opt/trn_rl_repo")

import ml_dtypes  # noqa: E402

import concourse.bass as bass  # noqa: E402  (kept for AP helpers)
import concourse.tile as tile  # noqa: E402
from concourse import bacc, mybir  # noqa: E402
from concourse.bass import ts  # noqa: E402

F32 = mybir.dt.float32
BF16 = mybir.dt.bfloat16
FP16 = mybir.dt.float16
AF = mybir.ActivationFunctionType

B, L, D, H = 2, 2048, 1024, 16
DK = D // H          # 64
HG = 4               # heads per core
DH = HG * DK         # 256 features per core
N_CORES = 8
KC = D // 128        # 8 contraction chunks for projections
T1C = 4              # number of 512-wide query chunks
T2C = 16             # number of 128-wide key chunks


def build_kernel(p_dtype=FP16):
    """Build the per-core Tile kernel (same program on all 8 cores)."""
    nc = bacc.Bacc(
        "TRN2",
        target_bir_lowering=False,
        debug=False,
        enable_asserts=False,
        num_devices=N_CORES,
    )

    qT = nc.dram_tensor("qT", [D, L], FP16, kind="ExternalInput").ap()
    kT = nc.dram_tensor("kT", [D, L], FP16, kind="ExternalInput").ap()
    vT = nc.dram_tensor("vT", [D, L], FP16, kind="ExternalInput").ap()
    wq = nc.dram_tensor("wq", [D, DH], FP16, kind="ExternalInput").ap()
    wk = nc.dram_tensor("wk", [D, DH], FP16, kind="ExternalInput").ap()
    wv = nc.dram_tensor("wv", [D, DH], FP16, kind="ExternalInput").ap()
    wo = nc.dram_tensor("wo", [DH, D], FP16, kind="ExternalInput").ap()
    cosT = nc.dram_tensor("cosT", [128, L], FP16, kind="ExternalInput").ap()
    sinT = nc.dram_tensor("sinT", [128, L], FP16, kind="ExternalInput").ap()
    maskT = nc.dram_tensor("maskT", [L, L], FP16, kind="ExternalInput").ap()
    out = nc.dram_tensor("out", [L, D], FP16, kind="ExternalOutput").ap()

    # DRAM views: partition-major chunking of the contraction dim
    qT_c = qT.rearrange("(c p) n -> p c n", p=128)        # [128, 8, 2048]
    kT_c = kT.rearrange("(c p) n -> p c n", p=128)
    vT_c = vT.rearrange("(c p) n -> p c n", p=128)
    wq_c = wq.rearrange("(c p) n -> p c n", p=128)        # [128, 8, 256]
    wk_c = wk.rearrange("(c p) n -> p c n", p=128)
    wv_c = wv.rearrange("(c p) n -> p c n", p=128)
    wo_c = wo.rearrange("(c p) n -> p c n", p=128)        # [128, 2, 1024]
    maskT_c = maskT.rearrange("(c p) n -> p c n", p=128)  # [128, 16, 2048]
    out_c = out.rearrange("(t p) n -> p t n", p=128)      # [128, 16, 1024]

    with tile.TileContext(nc) as tc, contextlib.ExitStack() as top:
        persist = top.enter_context(tc.tile_pool(name="persist", bufs=1))
        oconst = top.enter_context(tc.tile_pool(name="oconst", bufs=1))

        # persistent activations
        QT_sb = [persist.tile([128, L], FP16, name=f"QTsb{i}", tag=f"QT{i}")
                 for i in range(2)]
        KT_sb = [persist.tile([128, L], FP16, name=f"KTsb{i}", tag=f"KT{i}")
                 for i in range(2)]
        # head-contiguous repack: tile p holds heads 2p (rows 0-63: x1+x2)
        # and 2p+1 (rows 64-127)
        QT_hc = [persist.tile([128, L], FP16, name=f"QThc{p}", tag=f"QThc{p}")
                 for p in range(2)]
        KT_hc = [persist.tile([128, L], FP16, name=f"KThc{p}", tag=f"KThc{p}")
                 for p in range(2)]
        V_aug = [persist.tile([128, T2C, DK + 1], p_dtype, name=f"Vaugsb{h}",
                              tag=f"Vaug{h}") for h in range(HG)]
        OT_sb = [persist.tile([128, L], FP16, name=f"OTsb{p}", tag=f"OT{p}")
                 for p in range(2)]
        wo_sb = oconst.tile([128, 2, D], FP16, name="wo_sb", tag="wo")
        nc.sync.dma_start(wo_sb[:], wo_c)

        # ---- phase B: projections + rope ----------------------------------
        with tc.tile_pool(name="pconst", bufs=1) as pconst, \
             tc.tile_pool(name="proj_psum", bufs=4, space="PSUM") as pp:

            wq_sb = pconst.tile([128, KC, DH], FP16, name="wq_sb", tag="wq")
            wk_sb = pconst.tile([128, KC, DH], FP16, name="wk_sb", tag="wk")
            wv_sb = pconst.tile([128, KC, DH], FP16, name="wv_sb", tag="wv")
            cos_h = pconst.tile([128, L], FP16, name="cos_h", tag="cos")
            sin_h = pconst.tile([128, L], FP16, name="sin_h", tag="sin")
            nc.sync.dma_start(wq_sb[:], wq_c)
            nc.sync.dma_start(wk_sb[:], wk_c)
            nc.sync.dma_start(wv_sb[:], wv_c)
            nc.sync.dma_start(cos_h[:], cosT)
            nc.sync.dma_start(sin_h[:], sinT)

            def qk_proj(xs, rt, xT_view, w_sb, dst):
                # feature-major projection [256, 2048] in 4 psum quadrants
                # quadrant (fh, th): feature-half fh (x1/x2), token-half th
                ps = [pp.tile([128, L // 2], F32, name=f"psq{fh}{th}",
                              tag="proj")
                      for fh in range(2) for th in range(2)]
                for kk in range(KC):
                    xt = xs.tile([128, L], FP16, name="xt", tag="xT")
                    nc.sync.dma_start(xt[:], xT_view[:, kk, :])
                    for fh in range(2):
                        for th in range(2):
                            p = ps[fh * 2 + th]
                            for n in range(2):
                                nc.tensor.matmul(
                                    p[:, ts(n, 512)],
                                    lhsT=w_sb[:, kk, ts(fh, 128)],
                                    rhs=xt[:, th * 1024 + n * 512:
                                           th * 1024 + (n + 1) * 512],
                                    start=(kk == 0),
                                    stop=(kk == KC - 1),
                                )
                # rope: dst0 = x0*c - x1*s ; dst1 = x1*c + x0*s
                # (ScalarE evacuates psum to fp16; DVE rope runs at 2x fp16)
                for th in range(2):
                    x0f = rt.tile([128, 1024], FP16, name="x0f", tag="x0f")
                    x1f = rt.tile([128, 1024], FP16, name="x1f", tag="x1f")
                    nc.scalar.copy(x0f[:], ps[th][:])
                    nc.scalar.copy(x1f[:], ps[2 + th][:])
                    c = cos_h[:, ts(th, 1024)]
                    s = sin_h[:, ts(th, 1024)]
                    x0c = rt.tile([128, 1024], FP16, name="x0c", tag="x0c")
                    x1s = rt.tile([128, 1024], FP16, name="x1s", tag="x1s")
                    x1c = rt.tile([128, 1024], FP16, name="x1c", tag="x1c")
                    x0s = rt.tile([128, 1024], FP16, name="x0s", tag="x0s")
                    nc.vector.tensor_mul(x0c[:], x0f[:], c)
                    nc.vector.tensor_mul(x1s[:], x1f[:], s)
                    nc.vector.tensor_mul(x1c[:], x1f[:], c)
                    nc.vector.tensor_mul(x0s[:], x0f[:], s)
                    nc.vector.tensor_sub(dst[0][:, ts(th, 1024)], x0c[:], x1s[:])
                    nc.vector.tensor_add(dst[1][:, ts(th, 1024)], x1c[:], x0s[:])

            with tc.tile_pool(name="xstream", bufs=3) as xs, \
                 tc.tile_pool(name="ropetmp", bufs=2) as rt:
                qk_proj(xs, rt, qT_c, wq_sb, QT_sb)
                qk_proj(xs, rt, kT_c, wk_sb, KT_sb)
                # repack into head-contiguous layout for K=64 score matmuls
                for hh in range(HG):
                    p_, j_ = divmod(hh, 2)
                    for half in range(2):
                        nc.vector.tensor_copy(
                            QT_hc[p_][64 * j_ + 32 * half:
                                      64 * j_ + 32 * half + 32, :],
                            QT_sb[half][32 * hh:32 * hh + 32, :])
                        nc.vector.tensor_copy(
                            KT_hc[p_][64 * j_ + 32 * half:
                                      64 * j_ + 32 * half + 32, :],
                            KT_sb[half][32 * hh:32 * hh + 32, :])

            # V: token-major [t, o]; evacuated per head into V_aug with an
            # extra all-ones column (the softmax-denominator trick).
            # All 8 vT chunks stay resident so each token-tile runs its
            # whole contraction as one psum accumulation group (one bank).
            for h in range(HG):
                nc.gpsimd.memset(V_aug[h][:, :, DK:DK + 1], 1.0)
            with tc.tile_pool(name="vstream", bufs=1) as xsv:
                vchunks = []
                for kk in range(KC):
                    xt = xsv.tile([128, L], FP16, name=f"vt{kk}", tag=f"vT{kk}")
                    nc.scalar.dma_start(xt[:], vT_c[:, kk, :])
                    vchunks.append(xt)
                for tt in range(16):
                    pv = pp.tile([128, DH], F32, name="pv", tag="proj")
                    for kk in range(KC):
                        nc.tensor.matmul(
                            pv[:],
                            lhsT=vchunks[kk][:, ts(tt, 128)],
                            rhs=wv_sb[:, kk, :],
                            start=(kk == 0),
                            stop=(kk == KC - 1),
                        )
                    for h in range(HG):
                        nc.scalar.copy(
                            V_aug[h][:, tt, 0:DK],
                            pv[:, h * DK:(h + 1) * DK],
                        )

        # ---- phase C: attention -------------------------------------------
        with tc.tile_pool(name="att_psum", bufs=1, space="PSUM") as apsum, \
             tc.tile_pool(name="mask", bufs=2) as mpool, \
             tc.tile_pool(name="pexp", bufs=3) as pe_pool, \
             tc.tile_pool(name="small", bufs=2) as small:

            def emit_normalize(p, t1, acc):
                # normalize: OT[j] = acc[j][0:64] / acc[j][64]
                for j in range(2):
                    sj = small.tile([1, 512], F32, name=f"s{j}", tag=f"sum{j}")
                    nc.vector.tensor_copy(sj[:], acc[j][DK:DK + 1, :])
                    rcj = small.tile([1, 512], F32, name=f"rc{j}",
                                     tag=f"rc{j}")
                    nc.vector.reciprocal_approx_fast(rcj[:], sj[:])
                    rbj = small.tile([DK, 512], F32, name=f"rb{j}",
                                     tag=f"rb{j}")
                    nc.gpsimd.partition_broadcast(rbj[:], rcj[:])
                    nc.vector.tensor_mul(
                        OT_sb[p][ts(j, DK), ts(t1, 512)],
                        acc[j][0:DK, :],
                        rbj[:],
                    )

            pending = None
            pending = None
            for t1 in range(T1C):
                mt = mpool.tile([128, T2C, 512], FP16, name="mt", tag="mask")
                nc.sync.dma_start(mt[:], maskT_c[:, :, ts(t1, 512)])
                for p in range(2):
                    acc = [apsum.tile([DK + 1, 512], F32, name=f"acc{j}",
                                      tag=f"acc{j}", bufs=2) for j in range(2)]

                    def scores_mm(t2, p=p, t1=t1):
                        psc = apsum.tile([128, 1024], F32, name="psc",
                                         tag="sc", bufs=2)
                        for j in range(2):
                            nc.tensor.matmul(
                                psc[:, ts(j, 512)],
                                lhsT=KT_hc[p][ts(j, 64), ts(t2, 128)],
                                rhs=QT_hc[p][ts(j, 64), ts(t1, 512)],
                                start=True, stop=True,
                                tile_position=(64 * j, 0),
                            )
                        return psc

                    psc = scores_mm(0)
                    # previous block's normalize lands after the next block's
                    # first score matmuls so the PE never idles past the HAM
                    # re-throttle window
                    if pending is not None:
                        emit_normalize(*pending)
                        pending = None
                    for t2 in range(T2C):
                        pex = pe_pool.tile([128, 1024], p_dtype, name="pex",
                                           tag="pex")
                        nc.scalar.activation(pex[:], psc[:], AF.Exp)
                        if t2 + 1 < T2C:
                            psc = scores_mm(t2 + 1)
                        pm = pe_pool.tile([128, 1024], p_dtype, name="pm",
                                          tag="pm")
                        nc.vector.tensor_mul(
                            pm[:], pex[:],
                            mt[:, t2, None, :].broadcast_to([128, 2, 512]))
                        for j in range(2):
                            nc.tensor.matmul(
                                acc[j],
                                lhsT=V_aug[2 * p + j][:, t2, :],
                                rhs=pm[:, ts(j, 512)],
                                start=(t2 == 0),
                                stop=(t2 == T2C - 1),
                            )
                    pending = (p, t1, acc)
            emit_normalize(*pending)

        # ---- phase D: output projection -----------------------------------
        with tc.tile_pool(name="o_psum", bufs=2, space="PSUM") as opsum, \
             tc.tile_pool(name="ostage", bufs=2) as ostage:
            for t in range(16):
                po = opsum.tile([128, D], F32, name="po", tag="po")
                for p in range(2):
                    for j in range(2):
                        nc.tensor.matmul(
                            po[:, ts(j, 512)],
                            lhsT=OT_sb[p][:, ts(t, 128)],
                            rhs=wo_sb[:, p, ts(j, 512)],
                            start=(p == 0),
                            stop=(p == 1),
                        )
                ob = ostage.tile([128, D], FP16, name="ob", tag="ob")
                nc.scalar.copy(ob[:], po[:])
                nc.sync.dma_start(out_c[:, t, :], ob[:])

    nc.compile()
    return nc


def shard_inputs(q, k, v, mask, w_q, w_k, w_v, w_o):
    q = np.asarray(q, np.float32)
    k = np.asarray(k, np.float32)
    v = np.asarray(v, np.float32)
    w_q = np.asarray(w_q, np.float32)
    w_k = np.asarray(w_k, np.float32)
    w_v = np.asarray(w_v, np.float32)
    w_o = np.asarray(w_o, np.float32)
    mask = np.asarray(mask)

    qT = [np.ascontiguousarray(q[b].T).astype(np.float16) for b in range(B)]
    kT = [np.ascontiguousarray(k[b].T).astype(np.float16) for b in range(B)]
    vT = [np.ascontiguousarray(v[b].T).astype(np.float16) for b in range(B)]
    maskT_bf = np.ascontiguousarray(mask[0, 0].T).astype(np.float16)

    inv = 1.0 / (10000.0 ** (np.arange(0, DK, 2) / DK))   # [32]
    t = np.arange(L)
    fr = np.outer(inv, t)                                 # [32, 2048]
    cos_tab = np.tile(np.cos(fr), (4, 1)).astype(np.float16)  # [128, 2048]
    sin_tab = np.tile(np.sin(fr), (4, 1)).astype(np.float16)

    even = np.arange(0, DK, 2)
    odd = np.arange(1, DK, 2)
    scale = 1.0 / np.sqrt(DK)

    in_maps = []
    for core in range(N_CORES):
        b, g = divmod(core, N_CORES // B)
        hs = [HG * g + i for i in range(HG)]
        rows_qk = np.concatenate([h * DK + even for h in hs]
                                 + [h * DK + odd for h in hs])
        rows_v = np.concatenate([np.arange(h * DK, (h + 1) * DK) for h in hs])
        in_maps.append({
            "qT": qT[b],
            "kT": kT[b],
            "vT": vT[b],
            "wq": np.ascontiguousarray((w_q[rows_qk, :] * scale).T).astype(np.float16),
            "wk": np.ascontiguousarray(w_k[rows_qk, :].T).astype(np.float16),
            "wv": np.ascontiguousarray(w_v[rows_v, :].T).astype(np.float16),
            "wo": np.ascontiguousarray(w_o[:, rows_v].T).astype(np.float16),
            "cosT": cos_tab,
            "sinT": sin_tab,
            "maskT": maskT_bf,
        })
    return in_maps


_compiled = None


def _get_compiled():
    global _compiled
    if _compiled is None:
        _compiled = build_kernel()
    return _compiled


def kernel(q, k, v, mask, w_q, w_k, w_v, w_o, _trace=False, _trace_cores=None):
    from concourse.bass_utils import run_bass_kernel_spmd

    nc = _get_compiled()
    in_maps = shard_inputs(q, k, v, mask, w_q, w_k, w_v, w_o)
    res = run_bass_kernel_spmd(
        nc, in_maps, core_ids=list(range(N_CORES)),
        trace=_trace, trace_cores=_trace_cores,
    )
    out = np.zeros((B, L, D), np.float32)
    for core in range(N_CORES):
        out[core // (N_CORES // B)] += res.results[core]["out"].astype(np.float32)
    kernel._last_results = res
    return out



# revision 14
# speedup vs baseline: 1.1317x; 1.1317x over previous
"""Trainium2 Bass kernel for nn_MultiHeadAttentionBlock (B=2, L=2048, D=1024, H=16).

Sharding: 8 cores = 2 batches x 4 head-groups (4 heads each), Megatron-style.
Each core computes q/k/v projections for its 4 heads (column-sharded weights),
RoPE, attention, and a partial output projection (row-sharded w_o). The host
sums the 4 partial outputs per batch (the "all-reduce").

v2: fully software-pipelined single schedule.
  - DMA order: wk+kT first, then wq+qT, tables+mask(t1=0), wv+vT, wo.
  - The attention phase keeps the Activation engine exp-only (exp of the
    full P matrix is the per-core floor at ~144us); all PSUM evacuations
    run on Pool/DVE.
  - attn@V runs "flipped": lhsT = P chunks [128k,128q], rhs = [V|1] so the
    PSUM accumulators are token-major [128q, 65] at full partition use --
    half the PE columns of the feature-major variant -- and softmax
    normalization becomes a per-partition reciprocal+scale (no partition
    broadcast). A PE transpose returns O to feature-major for w_o.
  - V projection is interleaved into attention blocks 0-1; the w_o
    projection of t1 is interleaved into blocks 2t1+2..3; outputs stream
    to DRAM per 128-token tile.
  - PSUM budget (8 banks): scores 2x[128,1024] (4) + acc 2x[128,512] (2)
    + aux ring 2x[128,512] f32 (2) shared by V-proj psums, w_o psums and
    the O-transpose target.
"""

import contextlib
import sys

import numpy as np

sys.path.insert(0, "/opt/trn_rl_repo")

import ml_dtypes  # noqa: E402

import concourse.bass as bass  # noqa: E402
import concourse.tile as tile  # noqa: E402
from concourse import bacc, mybir  # noqa: E402
from concourse.bass import ts  # noqa: E402

F32 = mybir.dt.float32
BF16 = mybir.dt.bfloat16
FP16 = mybir.dt.float16
AF = mybir.ActivationFunctionType

B, L, D, H = 2, 2048, 1024, 16
DK = D // H          # 64
HG = 4               # heads per core
DH = HG * DK         # 256 features per core
N_CORES = 8
KC = D // 128        # 8 contraction chunks for projections
T1C = 4              # number of 512-wide query chunks
T2C = 16             # number of 128-wide key chunks


def build_kernel():
    nc = bacc.Bacc(
        "TRN2",
        target_bir_lowering=False,
        debug=False,
        enable_asserts=False,
        num_devices=N_CORES,
    )

    qT = nc.dram_tensor("qT", [D, L], FP16, kind="ExternalInput").ap()
    kT = nc.dram_tensor("kT", [D, L], FP16, kind="ExternalInput").ap()
    vT = nc.dram_tensor("vT", [D, L], FP16, kind="ExternalInput").ap()
    wq = nc.dram_tensor("wq", [D, DH], FP16, kind="ExternalInput").ap()
    wk = nc.dram_tensor("wk", [D, DH], FP16, kind="ExternalInput").ap()
    wv = nc.dram_tensor("wv", [D, DH], FP16, kind="ExternalInput").ap()
    wo = nc.dram_tensor("wo", [DH, D], FP16, kind="ExternalInput").ap()
    cosT = nc.dram_tensor("cosT", [128, L], FP16, kind="ExternalInput").ap()
    sinT = nc.dram_tensor("sinT", [128, L], FP16, kind="ExternalInput").ap()
    maskT = nc.dram_tensor("maskT", [L, L], FP16, kind="ExternalInput").ap()
    ident = nc.dram_tensor("ident", [128, 128], F32, kind="ExternalInput").ap()
    out = nc.dram_tensor("out", [L, D], FP16, kind="ExternalOutput").ap()

    qT_c = qT.rearrange("(c p) n -> p c n", p=128)        # [128, 8, 2048]
    kT_c = kT.rearrange("(c p) n -> p c n", p=128)
    vT_c = vT.rearrange("(c p) n -> p c n", p=128)
    wq_c = wq.rearrange("(c p) n -> p c n", p=128)        # [128, 8, 256]
    wk_c = wk.rearrange("(c p) n -> p c n", p=128)
    wv_c = wv.rearrange("(c p) n -> p c n", p=128)
    wo_c = wo.rearrange("(c p) n -> p c n", p=128)        # [128, 2, 1024]
    maskT_c = maskT.rearrange("(c p) n -> p c n", p=128)  # [128, 16, 2048]
    out_c = out.rearrange("(t p) n -> p t n", p=128)      # [128, 16, 1024]

    with tile.TileContext(nc) as tc, contextlib.ExitStack() as top:
        persist = top.enter_context(tc.tile_pool(name="persist", bufs=1))
        mpool = top.enter_context(tc.tile_pool(name="mask", bufs=2))
        pmpool = top.enter_context(tc.tile_pool(name="pm", bufs=8))
        pepool = top.enter_context(tc.tile_pool(name="pex", bufs=4))
        oqpool = top.enter_context(tc.tile_pool(name="oq", bufs=4))
        ospool = top.enter_context(tc.tile_pool(name="ostage", bufs=2))
        smallp = top.enter_context(tc.tile_pool(name="small", bufs=4))
        vxpool = top.enter_context(tc.tile_pool(name="vx", bufs=1))

        # ---- persistent SBUF ------------------------------------------------
        KT_hc = [persist.tile([128, L], FP16, name=f"KThc{p}", tag=f"KThc{p}")
                 for p in range(2)]
        QT_hc = [persist.tile([128, L], FP16, name=f"QThc{p}", tag=f"QThc{p}")
                 for p in range(2)]
        V_all = persist.tile([128, T2C, HG, DK + 1], FP16, name="V_all",
                             tag="V_all")
        OT_sb = [persist.tile([128, L], FP16, name=f"OTsb{p}", tag=f"OT{p}")
                 for p in range(2)]
        wk_sb = persist.tile([128, KC, DH], FP16, name="wk_sb", tag="wk")
        wq_sb = persist.tile([128, KC, DH], FP16, name="wq_sb", tag="wq")
        wv_sb = persist.tile([128, KC, DH], FP16, name="wv_sb", tag="wv")
        wo_sb = persist.tile([128, 2, D], FP16, name="wo_sb", tag="wo")
        cos_h = persist.tile([128, L], FP16, name="cos_h", tag="cos")
        sin_h = persist.tile([128, L], FP16, name="sin_h", tag="sin")
        id_sb = persist.tile([128, 128], F32, name="id_sb", tag="ident")

        mt_tiles = {}

        # ---- phase B + C under shared transient scopes ----------------------
        with tc.tile_pool(name="xs", bufs=1) as xspool, \
             tc.tile_pool(name="pp", bufs=1, space="PSUM") as pp, \
             tc.tile_pool(name="rt", bufs=1) as rt, \
             tc.tile_pool(name="preroped", bufs=1) as prp:

            # ---- DMA preamble (order = priority) ----------------------------
            nc.sync.dma_start(wk_sb[:], wk_c)
            kxt = []
            for kk in range(KC):
                xt = xspool.tile([128, L], FP16, name=f"kx{kk}", tag=f"x{kk}")
                nc.sync.dma_start(xt[:], kT_c[:, kk, :])
                kxt.append(xt)
            nc.sync.dma_start(wq_sb[:], wq_c)
            nc.sync.dma_start(cos_h[:], cosT)
            nc.sync.dma_start(sin_h[:], sinT)
            mt_tiles[0] = mpool.tile([128, T2C, 512], FP16, name="mt0",
                                     tag="mask")
            nc.sync.dma_start(mt_tiles[0][:], maskT_c[:, :, ts(0, 512)])
            nc.sync.dma_start(wv_sb[:], wv_c)
            vxt = []
            for kk in range(KC):
                xt = vxpool.tile([128, L], FP16, name=f"vx{kk}", tag=f"v{kk}")
                nc.sync.dma_start(xt[:], vT_c[:, kk, :])
                vxt.append(xt)
            nc.sync.dma_start(wo_sb[:], wo_c)
            nc.sync.dma_start(id_sb[:], ident)
            nc.gpsimd.memset(V_all[:, :, :, DK:DK + 1], 1.0)

            # ---- phase B: projections + rope + repack -----------------------
            def proj_rope(xtiles, w_sb, dst0, dst1):
                ps = [pp.tile([128, 1024], F32, name=f"ps{q}", tag=f"ps{q}")
                      for q in range(4)]  # index fh*2+th
                for kk in range(KC):
                    for fh in range(2):
                        for th in range(2):
                            p_ = ps[fh * 2 + th]
                            for n in range(2):
                                nc.tensor.matmul(
                                    p_[:, ts(n, 512)],
                                    lhsT=w_sb[:, kk, ts(fh, 128)],
                                    rhs=xtiles[kk][:, th * 1024 + n * 512:
                                                   th * 1024 + (n + 1) * 512],
                                    start=(kk == 0),
                                    stop=(kk == KC - 1),
                                )
                # rope: dst0 = x0*c - x1*s ; dst1 = x1*c + x0*s
                for th in range(2):
                    x0f = rt.tile([128, 1024], FP16, name="x0f", tag="x0f")
                    x1f = rt.tile([128, 1024], FP16, name="x1f", tag="x1f")
                    nc.scalar.copy(x0f[:], ps[th][:])
                    nc.scalar.copy(x1f[:], ps[2 + th][:])
                    c = cos_h[:, ts(th, 1024)]
                    s = sin_h[:, ts(th, 1024)]
                    x0c = rt.tile([128, 1024], FP16, name="x0c", tag="x0c")
                    x1s = rt.tile([128, 1024], FP16, name="x1s", tag="x1s")
                    x1c = rt.tile([128, 1024], FP16, name="x1c", tag="x1c")
                    x0s = rt.tile([128, 1024], FP16, name="x0s", tag="x0s")
                    nc.vector.tensor_mul(x0c[:], x0f[:], c)
                    nc.vector.tensor_mul(x1s[:], x1f[:], s)
                    nc.vector.tensor_mul(x1c[:], x1f[:], c)
                    nc.vector.tensor_mul(x0s[:], x0f[:], s)
                    nc.vector.tensor_sub(dst0[:, ts(th, 1024)], x0c[:], x1s[:])
                    nc.vector.tensor_add(dst1[:, ts(th, 1024)], x1c[:], x0s[:])

            def repack(src0, src1, dst):
                # head-contiguous: dst[p][64j+32*half+..] <- src[half][32hh+..]
                for hh in range(HG):
                    p_, j_ = divmod(hh, 2)
                    for half, src in enumerate((src0, src1)):
                        nc.vector.tensor_copy(
                            dst[p_][64 * j_ + 32 * half:
                                    64 * j_ + 32 * half + 32, :],
                            src[32 * hh:32 * hh + 32, :])

            KT_sb0 = prp.tile([128, L], FP16, name="KTsb0", tag="pr0")
            KT_sb1 = prp.tile([128, L], FP16, name="KTsb1", tag="pr1")
            proj_rope(kxt, wk_sb, KT_sb0, KT_sb1)
            repack(KT_sb0, KT_sb1, KT_hc)

            qxt = []
            for kk in range(KC):
                xt = xspool.tile([128, L], FP16, name=f"qx{kk}", tag=f"x{kk}")
                nc.sync.dma_start(xt[:], qT_c[:, kk, :])
                qxt.append(xt)
            QT_sb0 = prp.tile([128, L], FP16, name="QTsb0", tag="pr0")
            QT_sb1 = prp.tile([128, L], FP16, name="QTsb1", tag="pr1")
            proj_rope(qxt, wq_sb, QT_sb0, QT_sb1)
            repack(QT_sb0, QT_sb1, QT_hc)

        # ---- phase C: pipelined attention blocks ----------------------------
        # block b = t1*2 + p; steps i = 0..15 per block:
        #   steps 0,1: tail of previous block's attn@V
        #   step 2: normalize(prev block) (DVE); step 3: PE transpose + evac
        #   steps lag..15: this block's attn@V, i2 = i - lag
        #   every step: scores(b,i) -> exp (ACT) -> mask-mul (DVE) -> pm ring
        #   blocks 0-1: V-projection interleave; blocks >=2: w_o projection
        with tc.tile_pool(name="att_psum", bufs=1, space="PSUM") as apsum, \
             tc.tile_pool(name="aux_psum", bufs=2, space="PSUM") as aux:

            def scores_mm(b, i):
                t1, p = b // 2, b % 2
                psc = apsum.tile([128, 1024], F32, name="psc", tag="psc",
                                 bufs=2)
                for j in range(2):
                    nc.tensor.matmul(
                        psc[:, ts(j, 512)],
                        lhsT=KT_hc[p][ts(j, 64), ts(i, 128)],
                        rhs=QT_hc[p][ts(j, 64), ts(t1, 512)],
                        start=True, stop=True,
                        tile_position=(64 * j, 0),
                    )
                return psc

            def exp_mask(b, i, psc):
                t1 = b // 2
                pex = pepool.tile([128, 1024], FP16, name="pex", tag="pex")
                nc.scalar.activation(pex[:], psc[:], AF.Exp)
                pm = pmpool.tile([128, 1024], FP16, name="pm", tag="pm")
                # every 4th mask-multiply runs on Pool (SBUF-only there) to
                # keep DVE below the ACT exp ceiling
                eng = nc.gpsimd if i % 4 == 3 else nc.vector
                eng.tensor_mul(
                    pm[:], pex[:],
                    mt_tiles[t1][:, i, None, :].broadcast_to([128, 2, 512]))
                return pm

            def attnv_mm(b, i2, pm, accs):
                # one psum accumulation group per acc bank: the 4 qc
                # sub-tiles share the bank's 2KB zero region
                p = b % 2
                for j in range(2):
                    for qc in range(4):
                        nc.tensor.matmul(
                            accs[j][:, qc * 128:qc * 128 + DK + 1],
                            lhsT=pm[:, j * 512 + qc * 128:
                                    j * 512 + (qc + 1) * 128],
                            rhs=V_all[:, i2, 2 * p + j, :],
                            start=(i2 == 0 and qc == 0),
                            stop=(i2 == T2C - 1 and qc == 3),
                        )

            def vproj(tt):
                pv = aux.tile([128, 512], F32, name="pv", tag="aux")
                for kk in range(KC):
                    nc.tensor.matmul(
                        pv[:, 0:DH],
                        lhsT=vxt[kk][:, ts(tt, 128)],
                        rhs=wv_sb[:, kk, :],
                        start=(kk == 0),
                        stop=(kk == KC - 1),
                    )
                nc.vector.tensor_copy(
                    V_all[:, tt, :, 0:DK],
                    pv[:, 0:DH].rearrange("p (h d) -> p h d", h=HG))

            def normalize(b, accs):
                # accs[j][:, qc*128 : qc*128+64] numerators (token-major),
                # col qc*128+64 the softmax denominator; scale by 1/denom
                # (per-partition scalar) into oq tiles [128q, 128dh].
                oqs = []
                for qc in range(4):
                    oq = oqpool.tile([128, 128], F32, name="oq", tag="oq")
                    for j in range(2):
                        rc = smallp.tile([128, 1], F32, name="rc", tag="rc")
                        nc.vector.reciprocal_approx_fast(
                            rc[:], accs[j][:, qc * 128 + DK:qc * 128 + DK + 1])
                        nc.vector.tensor_scalar_mul(
                            oq[:, ts(j, DK)],
                            accs[j][:, qc * 128:qc * 128 + DK], rc[:])
                    oqs.append(oq)
                return oqs

            def transpose_evac(b, oqs):
                t1, p = b // 2, b % 2
                tp = aux.tile([128, 512], F32, name="tp", tag="aux")
                for qc in range(4):
                    nc.tensor.matmul(
                        tp[:, ts(qc, 128)], lhsT=oqs[qc][:], rhs=id_sb[:],
                        is_transpose=True, start=True, stop=True,
                    )
                nc.vector.tensor_copy(OT_sb[p][:, ts(t1, 512)], tp[:])

            def outproj_chunk(t1o, ci):
                # ci in 0..7: t-tile = 4*t1o + ci//2, column half ci%2
                t = 4 * t1o + ci // 2
                jj = ci % 2
                po = aux.tile([128, 512], F32, name="po", tag="aux")
                for pp_ in range(2):
                    nc.tensor.matmul(
                        po[:],
                        lhsT=OT_sb[pp_][:, ts(t, 128)],
                        rhs=wo_sb[:, pp_, ts(jj, 512)],
                        start=(pp_ == 0),
                        stop=(pp_ == 1),
                    )
                ob = ospool.tile([128, 512], FP16, name="ob", tag="ob")
                nc.vector.tensor_copy(ob[:], po[:])
                nc.sync.dma_start(out_c[:, t, ts(jj, 512)], ob[:])

            pm_hist = {}      # (b, i) -> pm tile
            acc_hist = {}     # b -> accs
            oq_hist = {}      # b -> oq tiles
            LAGS = [4] + [2] * 7
            for b in range(8):
                t1, p = b // 2, b % 2
                accs = [apsum.tile([128, 512], F32, name=f"acc{j}",
                                   tag=f"acc{j}") for j in range(2)]
                acc_hist[b] = accs
                if p == 0 and t1 + 1 < T1C:
                    mt_tiles[t1 + 1] = mpool.tile([128, T2C, 512], FP16,
                                                  name=f"mt{t1+1}", tag="mask")
                    nc.sync.dma_start(mt_tiles[t1 + 1][:],
                                      maskT_c[:, :, ts(t1 + 1, 512)])
                lag = LAGS[b]
                for i in range(T2C):
                    # previous block's attn@V tail, spread over steps 0-1
                    if b >= 1 and i < 2:
                        pb, plag = b - 1, LAGS[b - 1]
                        n_per = plag // 2
                        for k in range(n_per):
                            i2 = T2C - plag + n_per * i + k
                            attnv_mm(pb, i2, pm_hist[(pb, i2)], acc_hist[pb])
                    if b >= 1 and i == 2:
                        oq_hist[b - 1] = normalize(b - 1, acc_hist[b - 1])
                        del acc_hist[b - 1]
                    if b >= 1 and i == 3:
                        transpose_evac(b - 1, oq_hist.pop(b - 1))
                    # this block's attn@V (lagged)
                    if i >= lag:
                        i2 = i - lag
                        attnv_mm(b, i2, pm_hist[(b, i2)], accs)
                    # scores -> exp -> mask
                    psc = scores_mm(b, i)
                    pm_hist[(b, i)] = exp_mask(b, i, psc)
                    # V projection interleave (block 0: tiles 0-12 at steps
                    # 3-15, tiles 13-15 doubled up on the last steps so the
                    # block-0 attn@V tail in block 1 finds them ready)
                    if b == 0 and 3 <= i:
                        vproj(i - 3)
                        if i >= 13:
                            vproj(i)
                    # w_o projection interleave (blocks >= 2)
                    if b >= 2:
                        t1o = (b - 2) // 2
                        if b % 2 == 0 and i in (8, 10, 12, 14):
                            outproj_chunk(t1o, (i - 8) // 2)
                        elif b % 2 == 1 and i in (2, 4, 6, 8):
                            outproj_chunk(t1o, 4 + (i - 2) // 2)

            # ---- tail: finish block 7, then t1=3 output projection ----------
            b, lag = 7, LAGS[7]
            for i2 in range(T2C - lag, T2C):
                attnv_mm(b, i2, pm_hist[(b, i2)], acc_hist[b])
            oqs = normalize(b, acc_hist[b])
            transpose_evac(b, oqs)
            for ci in range(8):
                outproj_chunk(3, ci)

    nc.compile()
    return nc


def shard_inputs(q, k, v, mask, w_q, w_k, w_v, w_o):
    q = np.asarray(q, np.float32)
    k = np.asarray(k, np.float32)
    v = np.asarray(v, np.float32)
    w_q = np.asarray(w_q, np.float32)
    w_k = np.asarray(w_k, np.float32)
    w_v = np.asarray(w_v, np.float32)
    w_o = np.asarray(w_o, np.float32)
    mask = np.asarray(mask)

    qT = [np.ascontiguousarray(q[b].T).astype(np.float16) for b in range(B)]
    kT = [np.ascontiguousarray(k[b].T).astype(np.float16) for b in range(B)]
    vT = [np.ascontiguousarray(v[b].T).astype(np.float16) for b in range(B)]
    maskT_bf = np.ascontiguousarray(mask[0, 0].T).astype(np.float16)

    inv = 1.0 / (10000.0 ** (np.arange(0, DK, 2) / DK))   # [32]
    t = np.arange(L)
    fr = np.outer(inv, t)                                 # [32, 2048]
    cos_tab = np.tile(np.cos(fr), (4, 1)).astype(np.float16)  # [128, 2048]
    sin_tab = np.tile(np.sin(fr), (4, 1)).astype(np.float16)
    ident = np.eye(128, dtype=np.float32)

    even = np.arange(0, DK, 2)
    odd = np.arange(1, DK, 2)
    scale = 1.0 / np.sqrt(DK)

    in_maps = []
    for core in range(N_CORES):
        b, g = divmod(core, N_CORES // B)
        hs = [HG * g + i for i in range(HG)]
        rows_qk = np.concatenate([h * DK + even for h in hs]
                                 + [h * DK + odd for h in hs])
        rows_v = np.concatenate([np.arange(h * DK, (h + 1) * DK) for h in hs])
        in_maps.append({
            "qT": qT[b],
            "kT": kT[b],
            "vT": vT[b],
            "wq": np.ascontiguousarray((w_q[rows_qk, :] * scale).T).astype(np.float16),
            "wk": np.ascontiguousarray(w_k[rows_qk, :].T).astype(np.float16),
            "wv": np.ascontiguousarray(w_v[rows_v, :].T).astype(np.float16),
            "wo": np.ascontiguousarray(w_o[:, rows_v].T).astype(np.float16),
            "cosT": cos_tab,
            "sinT": sin_tab,
            "maskT": maskT_bf,
            "ident": ident,
        })
    return in_maps


_compiled = None


def _get_compiled():
    global _compiled
    if _compiled is None:
        _compiled = build_kernel()
    return _compiled


def kernel(q, k, v, mask, w_q, w_k, w_v, w_o, _trace=False, _trace_cores=None):
    from concourse.bass_utils import run_bass_kernel_spmd

    nc = _get_compiled()
    in_maps = shard_inputs(q, k, v, mask, w_q, w_k, w_v, w_o)
    res = run_bass_kernel_spmd(
        nc, in_maps, core_ids=list(range(N_CORES)),
        trace=_trace, trace_cores=_trace_cores,
    )
    out = np.zeros((B, L, D), np.float32)
    for core in range(N_CORES):
        out[core // (N_CORES // B)] += res.results[core]["out"].astype(np.float32)
    kernel._last_results = res
    return out
